# revision 4
# baseline (speedup 1.0000x reference)
"""Trainium2 Bass kernel for the ConcreteLayer training forward pass.

Computes out = x @ softmax((weight - ln(-ln((1-tiny)*uniform + tiny))) / T, axis=1)

Strategy (8 NeuronCores, 4x2 grid):
  - 4 batch groups x 2 out-column halves; core = 2*p + q.
  - Chunk-pipelined softmax (ramped chunk sizes 2,2,4,...): Ln+Ln on
    scalar, (w - m) subtract on vector for the first two chunks then
    gpsimd, wide Exp on scalar, per-chunk 3D tensor_reduce row sums on
    vector.
  - Row sums exchanged between column-half siblings in fine-grained
    AllGather groups; each group's normalize (vector) + matmuls (tensor)
    run as soon as the CC result lands, overlapping the GEMM with
    softmax production.
  - xt (lhsT) loads are explicitly dependency-paced (add_dep_helper on
    each chunk's first Ln) so the tile scheduler cannot hoist them ahead
    of the latency-critical u/w loads.
  - GEMM: bf16 lhsT x normalized bf16 e, f32 PSUM, 8 banks.
  - Output stored as bf16 (upcast on host) to trim HBM store traffic.
"""

import sys

import numpy as np

for _p in ("/opt/trn_rl_repo",):
    if _p not in sys.path:
        sys.path.insert(0, _p)

B, IN, OUT = 4096, 4096, 1024
GB, GO = 4, 2  # batch groups x out-half groups
BS = B // GB  # 1024 batch rows per core
OH = OUT // GO  # 512 out cols per core
P = 128
KT = IN // P  # 32 contraction tiles
MBT = BS // P  # 8 output row tiles per core
NCORES = 8
CHUNKS = [2, 2, 4, 4, 4, 4, 4, 4, 4]  # ktiles per softmax chunk (sum 32)
VEC_SUB_CHUNKS = 2  # first N chunks do the subtract on vector (latency)
GROUPS = [2, 2, 4, 8, 8, 4, 4]  # ktiles per row-sum exchange group
XTC = 4  # ktiles per xt load chunk
TINY = float(np.finfo(np.float32).tiny)

_PROGRAM = None
LAST_RESULT = None


def _pin_act_tables():
    """Steer the act-table-load pass to one set (has both Ln and Exp) so the
    compiler emits one ACT_TABLE_LOAD instead of reloading per tile."""
    import concourse.mybir as mybir
    from concourse import bacc, hw_specs

    orig = hw_specs.get_activation_tables.__wrapped__
    target = "natural_log_exp_and_others"
    strip = {
        mybir.ActivationFunctionType.Ln,
        mybir.ActivationFunctionType.Exp,
    }

    def pinned(arch):
        tables = orig(arch)
        if target not in tables:
            return tables
        return {
            name: (set(fns) if name == target else {f for f in fns if f not in strip})
            for name, fns in tables.items()
        }

    bacc.get_activation_tables = pinned


def _build_program():
    import concourse.bass as bass
    import concourse.mybir as mybir
    import concourse.tile as tile
    from concourse import bacc
    from concourse.tile_rust import add_dep_helper
    from contextlib import ExitStack

    _pin_act_tables()

    f32 = mybir.dt.float32
    bf16 = mybir.dt.bfloat16
    Ln = mybir.ActivationFunctionType.Ln
    Exp = mybir.ActivationFunctionType.Exp
    Alu = mybir.AluOpType

    nc = bacc.Bacc(
        "TRN2", target_bir_lowering=False, debug=False, num_devices=NCORES
    )

    xt_d = nc.dram_tensor("xt", [IN, BS], bf16, kind="ExternalInput")
    wh_d = nc.dram_tensor("wh", [IN, OH], bf16, kind="ExternalInput")
    uh_d = nc.dram_tensor("uh", [IN, OH], f32, kind="ExternalInput")
    t_d = nc.dram_tensor("tt", [1], f32, kind="ExternalInput")
    out_d = nc.dram_tensor("out", [BS, OH], bf16, kind="ExternalOutput")

    replica_groups = [[0, 1], [2, 3], [4, 5], [6, 7]]
    NCH = len(CHUNKS)
    cbounds = []
    s = 0
    for csz in CHUNKS:
        cbounds.append((s, s + csz))
        s += csz
    assert s == KT
    NG = len(GROUPS)
    gbounds = []
    s = 0
    for gsz in GROUPS:
        gbounds.append((s, s + gsz))
        s += gsz
    assert s == KT
    NXT = KT // XTC

    with tile.TileContext(nc) as tc, ExitStack() as ctx:
        dram = ctx.enter_context(tc.tile_pool(name="dram", bufs=1, space="DRAM"))
        singles = ctx.enter_context(tc.tile_pool(name="singles", bufs=1))
        chunks2 = ctx.enter_context(tc.tile_pool(name="chunks2", bufs=2))
        chunks4 = ctx.enter_context(tc.tile_pool(name="chunks4", bufs=3))
        outp = ctx.enter_context(tc.tile_pool(name="outp", bufs=4))
        psum = ctx.enter_context(tc.tile_pool(name="psum", bufs=1, space="PSUM"))

        # 1/T broadcast to all partitions.
        t_sb = singles.tile([P, 1], f32)
        t_ap = t_d.ap()
        nc.sync.dma_start(
            out=t_sb, in_=bass.AP(tensor=t_ap.tensor, offset=0, ap=[[0, P], [1, 1]])
        )
        invt = singles.tile([P, 1], f32)
        nc.vector.reciprocal(invt, t_sb)

        zero_t = singles.tile([P, 1], f32)
        nc.vector.memset(zero_t, 0.0)
        tiny_t = singles.tile([P, 1], f32)
        nc.vector.memset(tiny_t, TINY)

        # Resident tensors.
        xt_all = singles.tile([P, KT, BS], bf16)
        e_all = singles.tile([P, KT, OH], bf16)
        sums = singles.tile([P, KT, 1], f32)
        invr = singles.tile([P, KT], f32)

        cc_in = [
            dram.tile([P, gsz], f32, name=f"cc_in{g}", tag=f"cc_in{g}")
            for g, gsz in enumerate(GROUPS)
        ]
        cc_out = [
            dram.tile([2, P, gsz], f32, name=f"cc_out{g}", tag=f"cc_out{g}")
            for g, gsz in enumerate(GROUPS)
        ]

        ps_tiles = [
            psum.tile([P, OH], f32, tag=f"ps{mb}", name=f"ps{mb}")
            for mb in range(MBT)
        ]

        ln_a = {}  # chunk idx -> first Ln instruction (xt pacing anchor)

        def chunk_front(kb):
            """u/w DMA + the two Ln passes for chunk kb."""
            ks, ke = cbounds[kb]
            csz = ke - ks
            pool = chunks2 if csz == 2 else chunks4
            u_t = pool.tile([P, csz, OH], f32, tag=f"u{csz}", name=f"u{csz}_t")
            w_t = pool.tile([P, csz, OH], bf16, tag=f"w{csz}", name=f"w{csz}_t")
            u_src = uh_d[ks * P : ke * P, :].rearrange("(g p) c -> p g c", p=P)
            w_src = wh_d[ks * P : ke * P, :].rearrange("(g p) c -> p g c", p=P)
            nc.sync.dma_start(out=u_t, in_=u_src)
            nc.scalar.dma_start(out=w_t, in_=w_src)
            # v = ln((1 - tiny)*u + tiny)            (negative)
            ln_a[kb] = nc.scalar.activation(
                u_t, u_t, Ln, bias=tiny_t[:], scale=1.0 - TINY
            )
            # m = ln(-v) = -gumbel
            nc.scalar.activation(u_t, u_t, Ln, bias=zero_t[:], scale=-1.0)
            return u_t, w_t

        def chunk_back(kb, u_t, w_t):
            """sub (vector early / gpsimd later), wide Exp, row sums."""
            ks, ke = cbounds[kb]
            eng = nc.vector if kb < VEC_SUB_CHUNKS else nc.gpsimd
            eng.tensor_sub(u_t, w_t, u_t)
            nc.scalar.activation(
                e_all[:, ks:ke, :], u_t, Exp, bias=zero_t[:], scale=invt[:]
            )
            nc.vector.tensor_reduce(
                sums[:, ks:ke, :],
                e_all[:, ks:ke, :],
                mybir.AxisListType.X,
                Alu.add,
            )

        def xt_load(xb, pace_inst):
            """One 4-ktile chunk of the lhsT; issue paced behind pace_inst."""
            base = xb * XTC * P
            src = xt_d[base : base + XTC * P, :].rearrange("(g p) b -> p g b", p=P)
            inst = nc.gpsimd.dma_start(
                out=xt_all[:, xb * XTC : (xb + 1) * XTC, :], in_=src
            )
            if pace_inst is not None:
                add_dep_helper(inst.ins, pace_inst.ins, reason="pace xt load")

        def exchange(g):
            gs, ge = gbounds[g]
            nc.sync.dma_start(out=cc_in[g], in_=sums[:, gs:ge, 0])
            nc.gpsimd.collective_compute(
                "AllGather",
                Alu.bypass,
                replica_groups=replica_groups,
                ins=[cc_in[g].opt()],
                outs=[cc_out[g].opt()],
            )

        def finish(g):
            gs, ge = gbounds[g]
            gsz = ge - gs
            both = singles.tile([P, 2, gsz], f32, name=f"both{g}", tag=f"both{g}")
            nc.sync.dma_start(
                out=both, in_=cc_out[g][:].rearrange("g p k -> p g k")
            )
            tot = singles.tile([P, gsz], f32, name=f"tot{g}", tag=f"tot{g}")
            nc.vector.tensor_add(tot, both[:, 0, :], both[:, 1, :])
            nc.vector.reciprocal(invr[:, gs:ge], tot)
            for ki in range(gs, ge):
                nc.vector.tensor_scalar_mul(
                    e_all[:, ki, :], e_all[:, ki, :], invr[:, ki : ki + 1]
                )
            for ki in range(gs, ge):
                for mb in range(MBT):
                    nc.tensor.matmul(
                        ps_tiles[mb][:],
                        lhsT=xt_all[:, ki, mb * P : (mb + 1) * P],
                        rhs=e_all[:, ki, :],
                        start=(ki == 0),
                        stop=(ki == KT - 1),
                    )

        done_k = 0
        next_g = 0

        def maybe_exchange():
            nonlocal next_g
            while next_g < NG and gbounds[next_g][1] <= done_k:
                exchange(next_g)
                finish(next_g)
                next_g += 1

        next_xt = 0
        for kb in range(NCH):
            u_t, w_t = chunk_front(kb)
            # Pace xt: first two xt chunks behind chunk 0's Ln, then one per
            # compute chunk.
            if kb == 0:
                xt_load(0, ln_a[0])
                next_xt = 1
            elif next_xt < NXT:
                xt_load(next_xt, ln_a[kb])
                next_xt += 1
            chunk_back(kb, u_t, w_t)
            done_k = cbounds[kb][1]
            maybe_exchange()
        while next_xt < NXT:
            xt_load(next_xt, ln_a[NCH - 1])
            next_xt += 1
        assert next_g == NG

        # Drain PSUM (f32 -> bf16) and store.
        for mb in range(MBT):
            o_t = outp.tile([P, OH], bf16, tag="o")
            nc.vector.tensor_copy(o_t, ps_tiles[mb][:])
            nc.sync.dma_start(out=out_d[mb * P : (mb + 1) * P, :], in_=o_t)

    nc.compile()
    return nc


def kernel(x, weight, uniform, T):
    global _PROGRAM, LAST_RESULT
    import ml_dtypes
    from concourse.bass_utils import run_bass_kernel_spmd

    if _PROGRAM is None:
        _PROGRAM = _build_program()
    nc = _PROGRAM

    bf = ml_dtypes.bfloat16
    x = np.asarray(x, dtype=np.float32)
    weight = np.asarray(weight, dtype=np.float32)
    uniform = np.ascontiguousarray(np.asarray(uniform, dtype=np.float32))
    T = np.ascontiguousarray(np.asarray(T, dtype=np.float32)).reshape([1])

    xt = np.ascontiguousarray(x.T.astype(bf))  # [IN, B] bf16
    wb = weight.astype(bf)
    in_maps = []
    for c in range(NCORES):
        p, q = c // GO, c % GO
        in_maps.append(
            {
                "xt": np.ascontiguousarray(xt[:, p * BS : (p + 1) * BS]),
                "wh": np.ascontiguousarray(wb[:, q * OH : (q + 1) * OH]),
                "uh": np.ascontiguousarray(uniform[:, q * OH : (q + 1) * OH]),
                "tt": T,
            }
        )

    res = run_bass_kernel_spmd(nc, in_maps, core_ids=list(range(NCORES)))
    LAST_RESULT = res

    out = np.empty((B, OUT), dtype=np.float32)
    for c in range(NCORES):
        p, q = c // GO, c % GO
        out[p * BS : (p + 1) * BS, q * OH : (q + 1) * OH] = res.results[c][
            "out"
        ].astype(np.float32)
    return out
